# revision 51
# baseline (speedup 1.0000x reference)
"""Trainium2 Bass kernel for nn_MemoryBlock (batched LSTM scan with reset gating).

Problem (hardcoded shapes):
  bs=512, na=64, seq_len=16, nt=32, H=512, N_ATTN=256.
  x = concat(h_self[:,:,256:], h_inter, -1)            -> [512, 64, 512]
  time-major X: [16, 2048, 512]; LSTM cell per step with
  h,c reset-masked by (1-reset) before the cell. Outputs all
  intermediate h,c states, remapped back to [512, 64, 512].

Strategy: data-parallel over the 2048-row step-batch, 256 rows/core on 8
cores; weights replicated. Per core the batch splits into 2 independent
128-column streams so the recurrence latency of one stream hides behind the
other's engine work.

Matmuls run as fp8e4 DoubleRow (2 K-tiles of 128 per instruction, 0.5
cycles/row): weights are scaled x128 and split hi+residual; per-gate product
counts are chosen by error sensitivity (validated vs the fp32 reference,
rel err ~1.4e-2 < 2e-2):
  g (tanh, slope 1):   x_hi@W_hi + x_lo@W_hi + x_hi@W_lo   (3 products)
  f (sigmoid, mult.):  x_hi@W_hi + x_hi@W_lo                (2 products)
  i, o (sigmoid):      x_hi@W_hi                            (1 product)
  h-part (all gates):  h_fp8@W_hh_hi                        (1 product)
Residuals are stored unscaled (subnormal-heavy but only on small elements,
whose products are negligible). PSUM accumulates fp32; the x128 weight scale
is divided out by the activation's scale=1/128.

Gate order in PSUM is permuted to [g, i, f, o]; per stream and step the ACT
engine runs four instructions (tanh g-bank, sigmoid i+f banks, sigmoid
o-bank, tanh c) sized so the i/f evac unblocks the c-chain earliest. Cell
math and reset-mask muls are bf16 on DVE (2x_1p); o and c are pre-masked
with the next step's reset mask (om/cm) so the post-tanh critical path to
the next h-matmul is the single fp8 multiply hm = om*th. PSUM accumulation
order is exploited (commutative): within each bank, whichever product's
inputs arrive first carries start=True, the last carries stop; stream B's
tanh tail is software-pipelined one iteration behind so the in-order ACT
and DVE queues never idle on the other stream's chain. Outputs stream out
as bf16 (hys over the SWDGE ring, cys over HWDGE); the host converts back
to f32.

Layouts (per core), feature-major "T" = [feature-on-partition, batch]:
  wh   [128, 64, 2, 128] fp8: wh[p, 16*pair+mi, j, q] = A[128*(2pair+j)+p, 128*mi+q]
       A = 128 * [W_ih | W_hh].T with gate columns permuted to (g,i,f,o)
  wxlo [128, 16, 2, 128] fp8: same for the x-row residual of A, gates g,f only
  x8   [16, 128, 2, 4, 256] fp8: x hi/lo terms, x8[t, p, e, kc, b] = X_e[t, row b, 128kc+p]
  m2   [8, 128, 2, 256] bf16: (1-reset) replicated over partitions, step pairs
  h0, c0 [128, 4, 256] bf16: initial states, feature-major
  hys, cys [16, 2, 128, 4, 128] bf16: outputs, stream-major (host transposes back)
"""

import sys

import numpy as np

sys.path.insert(0, "/opt/pypackages")
sys.path.insert(0, "/opt/trn_rl_repo")

import concourse.bass as bass  # noqa: E402,F401
import concourse.bacc as bacc  # noqa: E402
import concourse.mybir as mybir  # noqa: E402
import concourse.tile as tile  # noqa: E402

SEQ = 16
NT = 32
NA = 64
H = 512
N_ATTN = 256
BS = NT * SEQ  # 512
BATCH = NT * NA  # 2048
N_CORES = 8
RPC = BATCH // N_CORES  # 256 rows per core
WS = 128.0  # weight pre-scale, divided out in the activation
F32 = mybir.dt.float32
BF16 = mybir.dt.bfloat16
FP8 = mybir.dt.float8e4
DR = mybir.MatmulPerfMode.DoubleRow

_CACHE = {}


def _build_bass():
    """Build the single-core Bass program (same NEFF runs SPMD on 8 cores)."""
    nc = bacc.Bacc(None, target_bir_lowering=False)

    wh_d = nc.dram_tensor("wh", [128, 64, 2, 128], FP8, kind="ExternalInput")
    wxlo_d = nc.dram_tensor("wxlo", [128, 16, 2, 128], FP8, kind="ExternalInput")
    x8_d = nc.dram_tensor("x8", [SEQ, 128, 2, 4, 256], FP8, kind="ExternalInput")
    m2_d = nc.dram_tensor("m2", [SEQ // 2, 128, 2, 256], BF16, kind="ExternalInput")
    h0_d = nc.dram_tensor("h0", [128, 4, 256], BF16, kind="ExternalInput")
    c0_d = nc.dram_tensor("c0", [128, 4, 256], BF16, kind="ExternalInput")
    hys_d = nc.dram_tensor("hys", [SEQ, 2, 128, 4, 128], BF16, kind="ExternalOutput")
    cys_d = nc.dram_tensor("cys", [SEQ, 2, 128, 4, 128], BF16, kind="ExternalOutput")

    SIG = mybir.ActivationFunctionType.Sigmoid
    TANH = mybir.ActivationFunctionType.Tanh

    with tile.TileContext(nc) as tc:
        with (
            tc.tile_pool(name="const", bufs=1) as const,
            tc.tile_pool(name="xin", bufs=4) as xin,
            tc.tile_pool(name="min", bufs=4) as min_,
            tc.tile_pool(name="state", bufs=2) as state,
            tc.tile_pool(name="gates", bufs=2) as gpool,
            tc.tile_pool(name="psum", bufs=1, space="PSUM") as psum,
        ):
            # --- preamble DMAs, in first-consumption order -----------------
            x_tiles, m_pairs = {}, {}

            def load_x(t):
                a = xin.tile([128, 2, 4, 256], FP8, tag="x8", name=f"x8_{t}")
                nc.sync.dma_start(a[:], x8_d[t])
                x_tiles[t] = a

            def load_m(pair):
                m = min_.tile([128, 2, 256], BF16, tag="m", name=f"m{pair}")
                nc.sync.dma_start(m[:], m2_d[pair])
                m_pairs[pair] = m

            def m_ap(t):
                return m_pairs[t // 2][:, t % 2]

            # preload both ACT function tables (sigmoid + tanh) on a scrap
            # tile so the 1.3us table loads overlap the weight DMA ramp
            scrap = state.tile([128, 1], BF16, tag="scrap", bufs=1)
            nc.vector.memset(scrap[:], 0.0)
            nc.scalar.activation(scrap[:], scrap[:],
                                 mybir.ActivationFunctionType.Sigmoid)
            nc.scalar.activation(scrap[:], scrap[:],
                                 mybir.ActivationFunctionType.Tanh)

            load_x(0)
            # weights: x-row pairs (0,1) first so step-0 x-matmuls start early
            wh = const.tile([128, 64, 2, 128], FP8, tag="wh", name="wh")
            wxlo = const.tile([128, 16, 2, 128], FP8, tag="wxlo", name="wxlo")
            nc.sync.dma_start(wh[:, 0:16], wh_d[:, 0:16])
            load_m(0)
            nc.sync.dma_start(wh[:, 16:32], wh_d[:, 16:32])
            nc.sync.dma_start(wxlo[:], wxlo_d[:])
            h0 = state.tile([128, 4, 256], BF16, tag="h_init", name="h0", bufs=1)
            c0 = state.tile([128, 4, 256], BF16, tag="c_init", name="c0", bufs=1)
            nc.gpsimd.dma_start(h0[:], h0_d[:])
            nc.gpsimd.dma_start(c0[:], c0_d[:])
            nc.sync.dma_start(wh[:, 32:48], wh_d[:, 32:48])
            nc.sync.dma_start(wh[:, 48:64], wh_d[:, 48:64])
            load_x(1)
            load_m(1)
            load_x(2)
            load_x(3)

            def lw(pair, mi):
                return wh[:, 16 * pair + mi]  # [128, 2, 128] fp8

            def lwx(pair, mi):
                # residual weights: g at slots 0-3, f at slots 4-7
                j = mi if mi < 4 else mi - 4
                return wxlo[:, 8 * pair + j]

            # --- per-step PSUM tiles & bank-group bookkeeping --------------
            # per stream, 4 banks: g (1), i+f (2), o (1). Finer tiles give
            # finer WAR deps so next-step x-matmuls start as each evac lands.
            def new_psum(t):
                tiles = []
                for s in range(2):
                    g = psum.tile([128, 4, 128], F32, tag=f"gps{s}",
                                  name=f"gps{t}_{s}")
                    pif = psum.tile([128, 8, 128], F32, tag=f"pif{s}",
                                    name=f"pif{t}_{s}")
                    po = psum.tile([128, 4, 128], F32, tag=f"po{s}",
                                   name=f"po{t}_{s}")
                    tiles.append((g, pif, po))
                return tiles

            def out_slot(tiles, s, mi):
                g, pif, po = tiles[s]
                if mi < 4:
                    return g[:, mi, :], (s, 0)
                if mi < 12:
                    return pif[:, mi - 4, :], (s, 1 + (mi - 4) // 4)
                return po[:, mi - 12, :], (s, 3)

            # --- matmul descriptor builders (flags assigned at flush) ------
            # x products for stream s over mi groups, bank-major (matching
            # the evac order that frees each bank), product-major inside
            def x_descs(tiles, xh_t, xl_t, s, mis):
                cols = slice(128 * s, 128 * (s + 1))
                plan = [(lw, 0, 0, None), (lw, 0, 1, None),
                        (lwx, 0, 0, "gf"), (lwx, 0, 1, "gf"),
                        (lw, 1, 0, "g"), (lw, 1, 1, "g")]
                out = []
                groups = [[mi for mi in mis if mi < 4],
                          [mi for mi in mis if 4 <= mi < 12],
                          [mi for mi in mis if mi >= 12]]
                for grp in groups:
                    for wfn, term, pair, filt in plan:
                        rhs_t = (xh_t, xl_t)[term]
                        for mi in grp:
                            if filt == "g" and mi >= 4:
                                continue
                            if filt == "gf" and not (mi < 4 or 8 <= mi < 12):
                                continue
                            o, bank = out_slot(tiles, s, mi)
                            out.append((o, wfn(pair, mi),
                                        rhs_t[:, 2 * pair : 2 * pair + 2, cols],
                                        bank))
                return out

            # h products for stream s, bank-major so each bank's accumulation
            # completes (stop lands) after only 8 matmuls, unlocking its evac
            def h_descs(tiles, hm, s):
                out = []
                for b in range(4):
                    for pair in (2, 3):
                        for mi in range(4 * b, 4 * b + 4):
                            o, bank = out_slot(tiles, s, mi)
                            out.append((o, lw(pair, mi),
                                        hm[s][:, 2 * (pair - 2) : 2 * pair - 2, :],
                                        bank))
                return out

            # emit matmuls: first write per bank gets start=True (tracked in
            # `started`, which may span multiple flushes of the same step);
            # last write per bank among `stop_banks` gets stop=True (PSUM
            # accumulation order is commutative).
            def flush_mms(descs, started=None, stop_banks=None):
                started = set() if started is None else started
                last = {}
                for i, (_, _, _, bank) in enumerate(descs):
                    if stop_banks is None or bank in stop_banks:
                        last[bank] = i
                for i, (o, lhsT, rhs, bank) in enumerate(descs):
                    st = bank not in started
                    started.add(bank)
                    nc.tensor.matmul(o, lhsT, rhs, start=st,
                                     stop=(last.get(bank) == i), perf_mode=DR)
                return started

            def m_bcast(t, s, chunks):
                return (m_ap(t)[:, 128 * s : 128 * (s + 1)]
                        .unsqueeze(1).broadcast_to([128, chunks, 128]))

            # t=0 reset-mask on the initial state (DVE)
            def mask0_s(s):
                hp = h0[:, :, 128 * s : 128 * (s + 1)]
                cp = c0[:, :, 128 * s : 128 * (s + 1)]
                hmv = state.tile([128, 4, 128], FP8, tag=f"hm{s}",
                                 name=f"hm0_{s}")
                cmv = state.tile([128, 4, 128], BF16, tag=f"cm{s}",
                                 name=f"cm0_{s}")
                nc.vector.tensor_mul(hmv[:], hp, m_bcast(0, s, 4))
                nc.vector.tensor_mul(cmv[:], cp, m_bcast(0, s, 4))
                return hmv, cmv

            # cell-evac stage for stream s: evacuate all gates, compute cn,
            # and pre-mask o (om) and cn (cm) with step-(t+1)'s reset mask so
            # the post-tanh critical path is a single multiply. `mid` is
            # emitted between the g and sif evacs (fills ACT with the other
            # stream's tanh while this stream's chain hasn't started).
            def evac_pre(t, tiles, cm, s, mid=None):
                g_ps, pif_ps, po_ps = tiles[s]
                gt = gpool.tile([128, 4, 128], BF16, tag=f"g{s}",
                                name=f"g{t}_{s}")
                nc.scalar.activation(gt[:], g_ps[:], TANH, scale=1.0 / WS)
                mid_out = mid() if mid is not None else None
                sif = gpool.tile([128, 8, 128], BF16, tag=f"sif{s}",
                                 name=f"sif{t}_{s}")
                nc.scalar.activation(sif[:], pif_ps[:], SIG, scale=1.0 / WS)
                so = gpool.tile([128, 4, 128], BF16, tag=f"so{s}",
                                name=f"so{t}_{s}")
                nc.scalar.activation(so[:], po_ps[:], SIG, scale=1.0 / WS)
                fcm = state.tile([128, 4, 128], BF16, tag=f"fcm{s}",
                                 name=f"fcm{t}_{s}")
                nc.vector.tensor_mul(fcm[:], sif[:, 4:8], cm[s][:])
                ig = state.tile([128, 4, 128], BF16, tag=f"ig{s}",
                                name=f"ig{t}_{s}")
                nc.vector.tensor_mul(ig[:], sif[:, 0:4], gt[:])
                cn = state.tile([128, 4, 128], BF16, tag=f"cn{s}",
                                name=f"c{t}_{s}")
                nc.vector.tensor_add(cn[:], ig[:], fcm[:])
                nc.sync.dma_start(cys_d[t, s], cn[:])
                om = cmn = None
                if t + 1 < SEQ:
                    om = state.tile([128, 4, 128], BF16, tag=f"om{s}",
                                    name=f"om{t}_{s}")
                    nc.vector.tensor_mul(om[:], so[:], m_bcast(t + 1, s, 4))
                    cmn = state.tile([128, 4, 128], BF16, tag=f"cm{s}",
                                     name=f"cm{t + 1}_{s}")
                    nc.vector.tensor_mul(cmn[:], cn[:], m_bcast(t + 1, s, 4))
                return so, cn, om, cmn, mid_out

            # post-tanh stage for stream s: tanh(c); hm(t+1) first (critical:
            # feeds next h-matmuls), then h for the output store.
            def cell_post(t, so, cn, om, s):
                th = state.tile([128, 4, 128], BF16, tag=f"th{s}",
                                name=f"th{t}_{s}")
                nc.scalar.activation(th[:], cn[:], TANH)
                hmn = None
                if om is not None:
                    hmn = state.tile([128, 4, 128], FP8, tag=f"hm{s}",
                                     name=f"hm{t + 1}_{s}")
                    nc.vector.tensor_mul(hmn[:], om[:], th[:])
                hn = state.tile([128, 4, 128], BF16, tag=f"hn{s}",
                                name=f"h{t}_{s}")
                nc.vector.tensor_mul(hn[:], so[:], th[:])
                # last step's stores ride the faster HWDGE gen (idle at tail)
                eng = nc.sync if t == SEQ - 1 else nc.gpsimd
                eng.dma_start(hys_d[t, s], hn[:])
                return hmn

            # --- t=0: masks on initial state -------------------------------
            # touch m0 with a 1-elem DVE copy so the first mask-mul carries
            # a single DMA sem wait (walrus allows one sync wait per instr).
            tch = state.tile([128, 1], BF16, tag="tch", bufs=1)
            nc.vector.tensor_copy(tch[:], m_pairs[0][:, 0, :1])
            cur_psum = new_psum(0)
            hm0a, cm0a = mask0_s(0)
            hm0b, cm0b = mask0_s(1)
            hm, cm = [hm0a, hm0b], [cm0a, cm0b]
            xt0 = x_tiles.pop(0)
            flush_mms(
                x_descs(cur_psum, xt0[:, 0], xt0[:, 1], 0, range(16))
                + x_descs(cur_psum, xt0[:, 0], xt0[:, 1], 1, range(16))
                + h_descs(cur_psum, hm, 0)
                + h_descs(cur_psum, hm, 1)
            )

            # Stream B's tanh/hm tail is software-pipelined one iteration
            # behind: th_B(t-1) is emitted after so_A(t) so it fills the ACT
            # slot exactly when cn_B(t-1) lands, and h_B(t)'s matmuls are
            # emitted right after hm_B(t) materializes.
            pend_b = None      # (t-1, so_b, cn_b, om_b)
            started_cur = None  # per-bank start bookkeeping for cur step
            for t in range(SEQ):
                mid = None
                if pend_b is not None:
                    pb = pend_b

                    def mid(pb=pb, t=t):
                        return cell_post(t - 1, pb[1], pb[2], pb[3], 1)
                so_a, cn_a, om_a, cm_a, hm_b = evac_pre(t, cur_psum, cm, 0,
                                                        mid=mid)
                if pend_b is not None:
                    hm[1] = hm_b
                    flush_mms(h_descs(cur_psum, hm, 1), started=started_cur)
                # stream-B's g evac before th_A on ACT
                g_ps, pif_ps, po_ps = cur_psum[1]
                gt_b = gpool.tile([128, 4, 128], BF16, tag="g1",
                                  name=f"g{t}_1")
                nc.scalar.activation(gt_b[:], g_ps[:], TANH, scale=1.0 / WS)
                hm_a = cell_post(t, so_a, cn_a, om_a, 0)
                # stream B evacs (g already emitted) + cell front half
                sif_b = gpool.tile([128, 8, 128], BF16, tag="sif1",
                                   name=f"sif{t}_1")
                nc.scalar.activation(sif_b[:], pif_ps[:], SIG, scale=1.0 / WS)
                so_b = gpool.tile([128, 4, 128], BF16, tag="so1",
                                  name=f"so{t}_1")
                nc.scalar.activation(so_b[:], po_ps[:], SIG, scale=1.0 / WS)
                fcm_b = state.tile([128, 4, 128], BF16, tag="fcm1",
                                   name=f"fcm{t}_1")
                nc.vector.tensor_mul(fcm_b[:], sif_b[:, 4:8], cm[1][:])
                ig_b = state.tile([128, 4, 128], BF16, tag="ig1",
                                  name=f"ig{t}_1")
                nc.vector.tensor_mul(ig_b[:], sif_b[:, 0:4], gt_b[:])
                cn_b = state.tile([128, 4, 128], BF16, tag="cn1",
                                  name=f"c{t}_1")
                nc.vector.tensor_add(cn_b[:], ig_b[:], fcm_b[:])
                nc.sync.dma_start(cys_d[t, 1], cn_b[:])
                om_b = cm_b = None
                if t + 1 < SEQ:
                    om_b = state.tile([128, 4, 128], BF16, tag="om1",
                                      name=f"om{t}_1")
                    nc.vector.tensor_mul(om_b[:], so_b[:], m_bcast(t + 1, 1, 4))
                    cm_b = state.tile([128, 4, 128], BF16, tag="cm1",
                                      name=f"cm{t + 1}_1")
                    nc.vector.tensor_mul(cm_b[:], cn_b[:], m_bcast(t + 1, 1, 4))
                pend_b = (t, so_b, cn_b, om_b)
                if t + 1 < SEQ:
                    hm, cm = [hm_a, None], [cm_a, cm_b]
                    if t + 4 < SEQ:
                        load_x(t + 4)
                    if t % 2 == 0 and (t + 4) // 2 < SEQ // 2:
                        load_m((t + 4) // 2)
                    nxt = new_psum(t + 1)
                    xt = x_tiles.pop(t + 1)
                    xh_t, xl_t = xt[:, 0], xt[:, 1]
                    # PE emission for step t+1 (h_B comes next iteration):
                    # stream A's x products, B's g-chunk x, A's h products,
                    # B's remaining x. Stops only for stream-A banks; B banks
                    # stop at their h products.
                    started_cur = flush_mms(
                        x_descs(nxt, xh_t, xl_t, 0, range(16))
                        + x_descs(nxt, xh_t, xl_t, 1, range(4))
                        + h_descs(nxt, hm, 0)
                        + x_descs(nxt, xh_t, xl_t, 1, range(4, 16)),
                        stop_banks={(0, b) for b in range(4)},
                    )
                    cur_psum = nxt
            # epilogue: finish stream B's last step
            cell_post(SEQ - 1, pend_b[1], pend_b[2], None, 1)

    nc.compile()
    return nc


def _get_nc():
    if "nc" not in _CACHE:
        _CACHE["nc"] = _build_bass()
    return _CACHE["nc"]


def _prep_inputs(h_self, h_inter, hxs, cxs, reset, W_ih, W_hh, b_ih, b_hh):
    """Host-side layout transforms -> list of per-core input dicts."""
    import ml_dtypes

    f = np.float32
    F8 = ml_dtypes.float8_e4m3
    bf16 = ml_dtypes.bfloat16

    x = np.concatenate([h_self[:, :, N_ATTN:], h_inter], axis=-1).astype(f)
    x_tm = np.ascontiguousarray(
        x.reshape(NT, SEQ, NA, H).transpose(1, 0, 2, 3).reshape(SEQ, BATCH, H)
    )
    resets = np.broadcast_to(reset.astype(f), (BS, NA))
    resets_tm = resets.reshape(NT, SEQ, NA).transpose(1, 0, 2).reshape(SEQ, BATCH)
    mask_tm = (1.0 - resets_tm).astype(f)
    h0 = hxs[::SEQ].reshape(BATCH, H).astype(f)
    c0 = cxs[::SEQ].reshape(BATCH, H).astype(f)

    assert not np.any(b_ih) and not np.any(b_hh), "nonzero LSTM bias unsupported"

    # A = 128 * [W_ih | W_hh].T [1024, 2048], gate columns permuted to g,i,f,o
    A = (np.concatenate([W_ih, W_hh], axis=1).T.astype(f) * WS)  # [1024, 2048]
    perm = np.concatenate([np.arange(2 * H, 3 * H),      # g
                           np.arange(0, H),              # i
                           np.arange(H, 2 * H),          # f
                           np.arange(3 * H, 4 * H)])     # o
    A = A[:, perm]
    A_hi8 = A.astype(F8)
    A_hi = A_hi8.astype(f)
    A_xlo = A[:H] - A_hi[:H]  # x rows residual
    # wh[p, 16*pair+mi, j, q] = A_hi[128*(2pair+j)+p, 128mi+q]
    wh = np.ascontiguousarray(
        A_hi8.reshape(4, 2, 128, 16, 128).transpose(2, 0, 3, 1, 4)
        .reshape(128, 64, 2, 128)
    )
    # wxlo: x pairs (0,1) x gate chunks g (0-3) and f (8-11)
    gf = np.concatenate([A_xlo[:, 0:512], A_xlo[:, 1024:1536]], axis=1)
    wxlo = np.ascontiguousarray(
        gf.astype(F8).reshape(2, 2, 128, 8, 128).transpose(2, 0, 3, 1, 4)
        .reshape(128, 16, 2, 128)
    )

    x_hi8 = x_tm.astype(F8)
    x_lo8 = (x_tm - x_hi8.astype(f)).astype(F8)

    def xlayout(a):  # [16, RPC rows, 512] -> [16, 128, 4, 256]
        return a.reshape(SEQ, RPC, 4, 128).transpose(0, 3, 2, 1)

    in_maps = []
    for cix in range(N_CORES):
        rows = slice(cix * RPC, (cix + 1) * RPC)
        x8 = np.ascontiguousarray(np.stack(
            [xlayout(x_hi8[:, rows, :]), xlayout(x_lo8[:, rows, :])], axis=1
        ).transpose(0, 2, 1, 3, 4))  # [16, 128, 2, 4, 256]
        m2 = np.ascontiguousarray(
            np.broadcast_to(mask_tm[:, rows][:, None, :], (SEQ, 128, RPC))
            .reshape(SEQ // 2, 2, 128, RPC).transpose(0, 2, 1, 3)
        ).astype(bf16)  # [8, 128, 2, 256]
        h0d = np.ascontiguousarray(
            h0[rows].reshape(RPC, 4, 128).transpose(2, 1, 0)).astype(bf16)
        c0d = np.ascontiguousarray(
            c0[rows].reshape(RPC, 4, 128).transpose(2, 1, 0)).astype(bf16)
        in_maps.append({"wh": wh, "wxlo": wxlo, "x8": x8,
                        "m2": m2, "h0": h0d, "c0": c0d})
    return in_maps


def _postprocess(results):
    """Per-core [16,2,128,4,128] bf16 outputs -> full [512, 64, 512] f32."""
    outs = []
    for key in ("hys", "cys"):
        tm = np.empty((SEQ, BATCH, H), dtype=np.float32)
        for cix, res in enumerate(results):
            rows = slice(cix * RPC, (cix + 1) * RPC)
            # res[t, s, p, kc, b'] -> tm[t, 128s+b', 128kc+p]
            r = np.asarray(res[key], dtype=np.float32)
            tm[:, rows, :] = (
                r.transpose(0, 1, 4, 3, 2).reshape(SEQ, RPC, H)
            )
        out = tm.reshape(SEQ, NT, NA, H).transpose(1, 0, 2, 3).reshape(BS, NA, H)
        outs.append(np.ascontiguousarray(out))
    return outs[0], outs[1]


def kernel(h_self, h_inter, hxs, cxs, reset, W_ih, W_hh, b_ih, b_hh, seq_len,
           trace=False, tmpdir=None):
    assert int(seq_len) == SEQ
    from concourse.bass_utils import run_bass_kernel_spmd

    nc = _get_nc()
    in_maps = _prep_inputs(
        np.asarray(h_self), np.asarray(h_inter), np.asarray(hxs), np.asarray(cxs),
        np.asarray(reset), np.asarray(W_ih), np.asarray(W_hh),
        np.asarray(b_ih), np.asarray(b_hh),
    )
    res = run_bass_kernel_spmd(
        nc, in_maps, core_ids=list(range(N_CORES)), trace=trace, tmpdir=tmpdir
    )
    _CACHE["last_results"] = res
    return _postprocess(res.results)
